# revision 1
# baseline (speedup 1.0000x reference)
"""Trainium2 Bass kernel for nn_DrawImageLayer (draw Gaussian strokes, max over time).

Reference semantics:
  out[b,i,j,0] = min(1, max_t I[b,t] * exp(-g*(r_i - y[b,t])^2) * exp(-g*(r_j - x[b,t])^2))
  r_k = k/28 - 0.5, g = (28/2)^2 = 196, shapes B=1024, T=64, canvas 28x28.

Strategy: pure data parallel — 128 batch rows per NeuronCore (= SBUF
partitions) across 8 cores. Compute in LOG domain so exp commutes with the
max and runs once on the final 784 pixels:
  out = exp( max_t [ (lnI[b,t] - g*dx[b,t,j]^2) - g*dy[b,t,i]^2 ] )
The min(.,1) clamp is dropped: I < 1 strictly => all log values < 0.

On this runtime every engine instruction costs ~25-50us nearly independent of
operand size — including standalone semaphore waits — so the kernel is built
from the fewest, largest ops (~11 instructions per core; every wait is
attached to its consumer instruction's sync_info via _wait_ge):
  DVE : d12 = r - [y|x]            one 3584-elem sub (halves via a concat-AP
                                   trick: offset dim [-1,2] flips channel)
        ex  = lnI - g*dx^2         one 1792-elem sub (lnI broadcast over j)
        cube[(i,j,t)] = ex - g*dy^2  two 25088-elem subs (image halves,
                                   3-free-dim APs with stride-0 broadcasts)
        reduce max over t          two segmented tensor_reduce (t innermost)
  ACT : Square(sqrt(g)*d12), Ln(I), final Exp(784)
"""

from contextlib import ExitStack

import numpy as np

import concourse.bass as bass
import concourse.mybir as mybir
from concourse.bass_utils import run_bass_kernel_spmd

SIZE = 28
T = 64
B = 1024
BC = 128  # batch rows per core
NCORES = 8
P2 = SIZE * SIZE
TI = T * SIZE  # 1792
G = (SIZE / 2.0) ** 2
SQRT_G = float(np.sqrt(G))
F32 = mybir.dt.float32
AO = mybir.AluOpType
AF = mybir.ActivationFunctionType
IH = SIZE // 2  # image rows per half-cube
CUBE = IH * SIZE * T  # 25088
RSOFF = T * 3  # grid columns appended after the (t,c) block
XCOLS = RSOFF + SIZE

_GRID = (np.arange(SIZE, dtype=np.float32) / SIZE - 0.5).astype(np.float32)


def _ap(t, offset, dims):
    """AP over an sbuf tensor: partition dim [row_pitch, 128] + free dims."""
    return bass.AP(t, offset, [[t.shape[1], BC]] + [list(d) for d in dims])


def build(rep: int = 1, drains: bool = False) -> bass.Bass:
    """One-core program, SPMD across 8 cores. rep>1 replicates the body
    (cumulative semaphore thresholds) for wall-clock delta timing."""
    nc = bass.Bass(detect_race_conditions=drains)
    xin = nc.declare_dram_parameter("xin", [BC, XCOLS], F32, isOutput=False)
    out = nc.declare_dram_parameter("out", [BC, P2], F32, isOutput=True)

    with ExitStack() as ctx:
        xs = ctx.enter_context(nc.sbuf_tensor([BC, XCOLS], F32))
        d12 = ctx.enter_context(nc.sbuf_tensor([BC, 2 * TI], F32))
        s12 = ctx.enter_context(nc.sbuf_tensor([BC, 2 * TI], F32))
        lnv = ctx.enter_context(nc.sbuf_tensor([BC, T], F32))
        ex = ctx.enter_context(nc.sbuf_tensor([BC, TI], F32))
        cube = ctx.enter_context(nc.sbuf_tensor([BC, CUBE], F32))
        img = ctx.enter_context(nc.sbuf_tensor([BC, P2], F32))
        dsx = ctx.enter_context(nc.semaphore("dsx"))  # xs in-dma
        dso = ctx.enter_context(nc.semaphore("dso"))  # out-dma
        va = ctx.enter_context(nc.semaphore("va"))  # vector -> scalar
        av = ctx.enter_context(nc.semaphore("av"))  # scalar -> vector
        vd = ctx.enter_context(nc.semaphore("vd"))  # scalar(exp) -> out dma
        block = ctx.enter_context(nc.Block())

        @block.sync
        def _(sync):
            for k in range(rep):
                di = sync.dma_start(out=xs[:, :], in_=xin[:, :])
                if k > 0:
                    di._wait_ge(av, 2 * k)  # prev Square+Ln done => xs consumed
                di.then_inc(dsx, 16)
                sync.dma_start(out=out[:, :], in_=img[:, :])._wait_ge(
                    vd, k + 1
                ).then_inc(dso, 16)
            sync.wait_ge(dsx, rep * 16)
            sync.wait_ge(dso, rep * 16)

        @block.vector
        def _(vector):
            for k in range(rep):
                # d12 = r - [y | x]  (y at channel 1, x at channel 0)
                nc.vector.tensor_tensor(
                    _ap(d12, 0, [[1, 2 * TI]]),
                    _ap(xs, RSOFF, [[0, 2], [0, T], [1, SIZE]]),
                    _ap(xs, 1, [[-1, 2], [3, T], [0, SIZE]]),
                    AO.subtract,
                )._wait_ge(dsx, k * 16 + 16).then_inc(va, 1)
                # ex[t*28+j] = lnI[t] - g*dx^2[t*28+j]
                nc.vector.tensor_tensor(
                    _ap(ex, 0, [[1, TI]]),
                    _ap(lnv, 0, [[1, T], [0, SIZE]]),
                    _ap(s12, TI, [[1, TI]]),
                    AO.subtract,
                )._wait_ge(av, 2 * k + 2)
                if drains:
                    vector.drain()
                for h in range(2):
                    if drains and h > 0:
                        vector.drain()
                    # cube[(i,j,t)] = ex[t*28+j] - g*dy^2[t*28+i], i in half h
                    nc.vector.tensor_tensor(
                        _ap(cube, 0, [[1, CUBE]]),
                        _ap(ex, 0, [[0, IH], [1, SIZE], [SIZE, T]]),
                        _ap(s12, h * IH, [[1, IH], [0, SIZE], [SIZE, T]]),
                        AO.subtract,
                    )
                    if drains:
                        vector.drain()
                    red = nc.vector.tensor_reduce(
                        _ap(img, h * IH * SIZE, [[1, IH * SIZE]]),
                        _ap(cube, 0, [[SIZE * T, IH], [T, SIZE], [1, T]]),
                        mybir.AxisListType.X,
                        AO.max,
                    )
                    if h == 0 and k > 0:
                        # WAR: prev body's out-dma must have drained img
                        red._wait_ge(dso, k * 16)
                red.then_inc(va, 1)

        @block.scalar
        def _(scalar):
            for k in range(rep):
                # Ln first: overlaps the DVE d12-sub (engines run concurrently)
                nc.scalar.activation(
                    _ap(lnv, 0, [[1, T]]),
                    _ap(xs, 2, [[3, T]]),
                    AF.Ln,
                )._wait_ge(dsx, k * 16 + 16).then_inc(av, 1)
                nc.scalar.activation(
                    _ap(s12, 0, [[1, 2 * TI]]),
                    _ap(d12, 0, [[1, 2 * TI]]),
                    AF.Square,
                    scale=SQRT_G,
                )._wait_ge(va, 2 * k + 1).then_inc(av, 1)
                nc.scalar.activation(
                    _ap(img, 0, [[1, P2]]),
                    _ap(img, 0, [[1, P2]]),
                    AF.Exp,
                )._wait_ge(va, 2 * k + 2).then_inc(vd, 1)

    return nc


def make_in_maps(x: np.ndarray) -> list:
    """Shard x (1024, 64, 3) -> per-core maps; grid constant appended."""
    maps = []
    for c in range(NCORES):
        xc = x[c * BC : (c + 1) * BC].reshape(BC, T * 3).astype(np.float32)
        xc = np.concatenate([xc, np.broadcast_to(_GRID, (BC, SIZE))], axis=1)
        maps.append({"xin": np.ascontiguousarray(xc)})
    return maps


def kernel(x: np.ndarray) -> np.ndarray:
    """Full inputs in, full output out: (1024, 64, 3) f32 -> (1024, 28, 28, 1) f32."""
    x = np.asarray(x, dtype=np.float32)
    assert x.shape == (B, T, 3), x.shape
    nc = build(rep=1)
    res = run_bass_kernel_spmd(nc, make_in_maps(x), list(range(NCORES)))
    outs = [res.results[c]["out"].reshape(BC, SIZE, SIZE, 1) for c in range(NCORES)]
    return np.concatenate(outs, axis=0)



# revision 10
# speedup vs baseline: 1.8148x; 1.8148x over previous
"""Trainium2 Bass kernel for nn_DrawImageLayer (draw Gaussian strokes, max over time).

Reference semantics:
  out[b,i,j,0] = min(1, max_t I[b,t] * exp(-g*(r_i - y[b,t])^2) * exp(-g*(r_j - x[b,t])^2))
  r_k = k/28 - 0.5, g = (28/2)^2 = 196, shapes B=1024, T=64, canvas 28x28.

Strategy: pure data parallel, 128 batch rows per NeuronCore (= SBUF
partitions) across 8 cores. Log domain so exp commutes with max:
  out = exp( max_t [ (lnI[t] + q_x[t,j]) + q_y[t,i] ] ),  q = -g*(r-coord)^2
The min(.,1) clamp is dropped: I < 1 strictly => all log values < 0.

Cost model measured on this runtime: DVE/ACT instructions carry a large fixed
cost (~30-80us) nearly independent of element count; GPSIMD (Pool) ops have
no such fixed cost (~2.8ns/elem); DMA-in ~2.5us, DMA-out ~75us fixed;
tensor_reduce cost grows with output-segment count (2x392 segments beats
1x784). Hence: few, large instructions; cheap prep on GPSIMD; single fused
fp16 cube; two segmented reduces; one activation; one DMA each way.

Per rep (per core), instruction list:
  sync : dma-in xs[128,220] (y|x|lnI|r, t innermost)      ~2.5us
  gps  : d12[c,k,t] = r_k - coord_c[t]      TT   3584     ~10us
         q = (d12 * -g) * d12  (stt, in-place)  3584      ~10us
         ex[j,t] = lnI[t] + q_x[j,t]        TT   1792     ~5us
  dve  : cube[i,j,t] = ex[j,t] + q_y[i,t]   TT  50176 f16 ~30us
         img[i,j] = max_t cube  (2 reduces, 392 segs each) ~80us
  act  : imgo = Exp(img)                         784      ~40us
  sync : dma-out imgo -> out[128,784]                     ~75us
Cross-rep overlap via per-edge counting semaphores; reps pipeline so the
DVE (~110us) and out-DMA (~75us) overlap.
"""

from contextlib import ExitStack

import numpy as np

import concourse.bass as bass
import concourse.mybir as mybir
from concourse.bass_utils import run_bass_kernel_spmd

SIZE = 28
T = 64
B = 1024
BC = 128  # batch rows per core
NCORES = 8
P2 = SIZE * SIZE
G = (SIZE / 2.0) ** 2
F32 = mybir.dt.float32
F16 = mybir.dt.float16
AO = mybir.AluOpType
AF = mybir.ActivationFunctionType

XCOLS = 3 * T + SIZE  # y(64) | x(64) | lnI(64) | r(28)
CUBE = P2 * T  # 50176, layout (i, j, t), t innermost
IH = SIZE // 2  # image rows per reduce (14 -> 392 output segments)

_GRID = (np.arange(SIZE, dtype=np.float32) / SIZE - 0.5).astype(np.float32)


def _ap(t, offset, dims):
    """AP over an sbuf tensor: partition dim [row_pitch, 128] + free dims."""
    return bass.AP(t, offset, [[t.shape[1], BC]] + [list(d) for d in dims])


def build(rep: int = 1) -> bass.Bass:
    nc = bass.Bass()
    xin = nc.declare_dram_parameter("xin", [BC, XCOLS], F32, isOutput=False)
    out = nc.declare_dram_parameter("out", [BC, P2], F32, isOutput=True)

    with ExitStack() as ctx:
        xs = ctx.enter_context(nc.sbuf_tensor([BC, XCOLS], F32))
        d12 = ctx.enter_context(nc.sbuf_tensor([BC, 2 * T * SIZE], F32))  # (c,k,t)
        ex = ctx.enter_context(nc.sbuf_tensor([BC, SIZE * T], F32))  # (j,t)
        cube = ctx.enter_context(nc.sbuf_tensor([BC, CUBE], F16))  # (i,j,t)
        img = ctx.enter_context(nc.sbuf_tensor([BC, P2], F32))
        imgo = ctx.enter_context(nc.sbuf_tensor([BC, 2 * P2], F32))  # double-buffered
        dsx = ctx.enter_context(nc.semaphore("dsx"))  # in-dma done
        gex = ctx.enter_context(nc.semaphore("gex"))  # gpsimd ex done
        vcb = ctx.enter_context(nc.semaphore("vcb"))  # dve cube done
        vrd = ctx.enter_context(nc.semaphore("vrd"))  # dve reduces done
        aex = ctx.enter_context(nc.semaphore("aex"))  # act exp done
        dso = ctx.enter_context(nc.semaphore("dso"))  # out-dma done
        block = ctx.enter_context(nc.Block())

        @block.sync
        def _(sync):
            for k in range(rep):
                di = sync.dma_start(out=xs[:, :], in_=xin[:, :])
                if k > 0:
                    # cube(k-1) done => gpsimd ex(k-1) done => xs consumed;
                    # also d12 (=q) free for rewrite
                    di._wait_ge(vcb, k)
                di.then_inc(dsx, 16)
                sync.dma_start(
                    out=out[:, :], in_=_ap(imgo, (k % 2) * P2, [[1, P2]])
                )._wait_ge(aex, k + 1).then_inc(dso, 16)
            sync.wait_ge(dsx, rep * 16)
            sync.wait_ge(dso, rep * 16)

        @block.gpsimd
        def _(gpsimd):
            for k in range(rep):
                # d12[(c,k,t)] = sqrt(g)*(r_k - coord_c[t])  (host pre-scales)
                nc.gpsimd.tensor_tensor(
                    _ap(d12, 0, [[1, 2 * T * SIZE]]),
                    _ap(xs, 3 * T, [[0, 2], [1, SIZE], [0, T]]),
                    _ap(xs, 0, [[T, 2], [0, SIZE], [1, T]]),
                    AO.subtract,
                )._wait_ge(dsx, k * 16 + 16)
                # q = d12 * d12 = g*(r-coord)^2, in place
                nc.gpsimd.tensor_tensor(
                    _ap(d12, 0, [[1, 2 * T * SIZE]]),
                    _ap(d12, 0, [[1, 2 * T * SIZE]]),
                    _ap(d12, 0, [[1, 2 * T * SIZE]]),
                    AO.mult,
                )
                # ex[(j,t)] = lnI[t] - q_x[(j,t)]
                nc.gpsimd.tensor_tensor(
                    _ap(ex, 0, [[1, SIZE * T]]),
                    _ap(xs, 2 * T, [[0, SIZE], [1, T]]),
                    _ap(d12, T * SIZE, [[T, SIZE], [1, T]]),
                    AO.subtract,
                ).then_inc(gex, 1)

        @block.vector
        def _(vector):
            for k in range(rep):
                # cube[(i,j,t)] = ex[(j,t)] - q_y[(i,t)]
                nc.vector.tensor_tensor(
                    _ap(cube, 0, [[1, CUBE]]),
                    _ap(ex, 0, [[0, SIZE], [T, SIZE], [1, T]]),
                    _ap(d12, 0, [[T, SIZE], [0, SIZE], [1, T]]),
                    AO.subtract,
                )._wait_ge(gex, k + 1).then_inc(vcb, 1)
                for h in range(2):
                    red = nc.vector.tensor_reduce(
                        _ap(img, h * IH * SIZE, [[1, IH * SIZE]]),
                        _ap(cube, h * IH * SIZE * T, [[SIZE * T, IH], [T, SIZE], [1, T]]),
                        mybir.AxisListType.X,
                        AO.max,
                    )
                    if h == 0 and k > 0:
                        # WAR: Exp(k-1) must have read img
                        red._wait_ge(aex, k)
                    if h == 1 and k > 1:
                        # WAR (2-rep slack via imgo double buffer): out-dma(k-2)
                        # must have read imgo[k%2] before Exp(k) rewrites it;
                        # red_h1 -> vrd -> Exp orders it.
                        red._wait_ge(dso, (k - 1) * 16)
                red.then_inc(vrd, 1)

        @block.scalar
        def _(scalar):
            for k in range(rep):
                nc.scalar.activation(
                    _ap(imgo, (k % 2) * P2, [[1, P2]]),
                    _ap(img, 0, [[1, P2]]),
                    AF.Exp,
                )._wait_ge(vrd, k + 1).then_inc(aex, 1)

    return nc


def make_in_maps(x: np.ndarray) -> list:
    """Shard x (1024, 64, 3) -> per-core host-prepped maps.

    Per core [128, 220] fp32: y[t] | x[t] | ln(I[t]) | grid r, t innermost.
    """
    x = np.asarray(x, dtype=np.float32)
    maps = []
    sg = np.float32(np.sqrt(G))
    with np.errstate(divide="ignore"):
        lnI = np.log(x[:, :, 2]).astype(np.float32)  # (B, T); -inf ok
    for c in range(NCORES):
        sl = slice(c * BC, (c + 1) * BC)
        xc = np.empty((BC, XCOLS), np.float32)
        xc[:, 0:T] = sg * x[sl, :, 1]  # sqrt(g)*y
        xc[:, T : 2 * T] = sg * x[sl, :, 0]  # sqrt(g)*x
        xc[:, 2 * T : 3 * T] = lnI[sl]
        xc[:, 3 * T :] = sg * _GRID[None, :]
        maps.append({"xin": np.ascontiguousarray(xc)})
    return maps


def kernel(x: np.ndarray) -> np.ndarray:
    """Full inputs in, full output out: (1024, 64, 3) f32 -> (1024, 28, 28, 1) f32."""
    x = np.asarray(x, dtype=np.float32)
    assert x.shape == (B, T, 3), x.shape
    nc = build(rep=1)
    res = run_bass_kernel_spmd(nc, make_in_maps(x), list(range(NCORES)))
    outs = [res.results[c]["out"].reshape(BC, SIZE, SIZE, 1) for c in range(NCORES)]
    return np.concatenate(outs, axis=0)
